# revision 5
# baseline (speedup 1.0000x reference)
"""Dice metric kernel for Trainium2 (Bass/Tile), 8-core data parallel.

Reference computation (per sample b):
    pred = argmax_c logits[b, :, h, w]   (softmax is monotonic -> argmax)
    For classes c = 1..7:
        tps_c  = #{pred == c and tgt == c}
        pmc_c  = #{pred == c},  tmc_c = #{tgt == c}
        dice_c = 2*tps_c / (pmc_c + tmc_c + 1e-5)
    out[b] = mean_c dice_c

Device design (v2): the host supplies a one-hot fp8 encoding of the target
(classes 1..7), which moves all per-class masking off the half-rate
scalar_tensor_tensor path:
  - DVE: max tree (3 batched tensor_max), pm = is_ge(x[1:8], mx) (1 op),
    prod = pm * tm_f16 (1 op). All full-rate tensor_tensor.
  - ACT: tm fp8 -> fp16 convert via Relu, fused accum gives tmc directly.
  - PE:  pmc and tps as ones-style matmul column sums of pm / prod,
    PSUM-accumulated across 512-col blocks and chunks.

Sharding: batch 16 -> 2 samples per core on 8 cores; host concatenates.
"""

import numpy as np
import ml_dtypes

import concourse.bacc as bacc
import concourse.mybir as mybir
import concourse.tile as tile
from concourse.bass_utils import run_bass_kernel_spmd

B, C, H, W = 16, 8, 512, 512
NCORES = 8
BPC = B // NCORES          # samples per core
P = 128                    # SBUF partitions
F = (H * W) // P           # free dim per plane (2048)
EPS = 1e-5

_f32 = mybir.dt.float32
_f16 = mybir.dt.float16
_f8 = mybir.dt.float8e4
_alu = mybir.AluOpType
_act = mybir.ActivationFunctionType


def _build_nc():
    nc = bacc.Bacc(None, target_bir_lowering=False, debug=False)
    x_dram = nc.dram_tensor("x", [BPC, C, P, F], _f16, kind="ExternalInput")
    m_dram = nc.dram_tensor("m", [BPC, 7, P, F], _f8, kind="ExternalInput")
    o_dram = nc.dram_tensor("o", [1, BPC], _f32, kind="ExternalOutput")
    i7_dram = nc.dram_tensor("i7", [8, 8], _f32, kind="ExternalInput")

    with tile.TileContext(nc) as tc:
        with (
            tc.tile_pool(name="xp", bufs=2) as xp,
            tc.tile_pool(name="mt", bufs=2) as mtp,
            tc.tile_pool(name="wk", bufs=2) as wk,
            tc.tile_pool(name="ac", bufs=2) as acp,
            tc.tile_pool(name="cst", bufs=1) as cst,
            tc.tile_pool(name="ps", bufs=2, space="PSUM") as ps,
        ):
            ones32 = cst.tile([P, 1], _f32)
            nc.gpsimd.memset(ones32[:], 1.0)
            kbias0 = cst.tile([P, 1], _f32)
            nc.gpsimd.memset(kbias0[:], 0.0)
            kbias1 = cst.tile([P, 1], _f32)
            nc.gpsimd.memset(kbias1[:], 1.0)
            ecs = cst.tile([P, 7, 8], _f16)
            nc.gpsimd.memset(ecs[:], 0.0)
            for ci in range(7):
                nc.gpsimd.memset(ecs[:, ci, ci : ci + 1], 1.0)
            osb = cst.tile([1, BPC], _f32)
            i7 = cst.tile([8, 8], _f32)

            for b in range(BPC):
                BOUNDS = [0, 512, 1024, F] if b == 0 else [0, 1024, F]
                NH = len(BOUNDS) - 1
                NBLK = F // 512  # total 512-col blocks per sample

                xr = x_dram[b].rearrange("c p f -> p c f")
                mr = m_dram[b].rearrange("c p f -> p c f")

                acc = acp.tile([P, 48], _f32, tag="acc")
                nc.gpsimd.memset(acc[:], 0.0)
                ppQ = ps.tile([8, 512], _f32, tag="ppQ")

                nmm = 0  # matmul counter for start/stop flags
                for h in range(NH):
                    hs = slice(BOUNDS[h], BOUNDS[h + 1])
                    Fh = BOUNDS[h + 1] - BOUNDS[h]
                    xb = xp.tile([P, C, Fh], _f16, tag="x")
                    tm8 = xp.tile([P, 7, Fh], _f8, tag="tm8")
                    l1 = mtp.tile([P, 4, Fh], _f16, tag="l1")

                    nc.sync.dma_start(tm8[:], mr[:, 0:7, hs])
                    if b == 0 and h == 0:
                        nc.sync.dma_start(xb[:, 0:2, :], xr[:, 0:2, hs])
                        nc.sync.dma_start(xb[:, 4:6, :], xr[:, 4:6, hs])
                        nc.sync.dma_start(xb[:, 2:4, :], xr[:, 2:4, hs])
                        nc.sync.dma_start(xb[:, 6:8, :], xr[:, 6:8, hs])
                        nc.sync.dma_start(i7[:], i7_dram[:])
                        nc.vector.tensor_max(
                            l1[:, 0:2, :], xb[:, 0:2, :], xb[:, 4:6, :]
                        )
                        nc.vector.tensor_max(
                            l1[:, 2:4, :], xb[:, 2:4, :], xb[:, 6:8, :]
                        )
                    else:
                        nc.sync.dma_start(xb[:, 0:4, :], xr[:, 0:4, hs])
                        nc.sync.dma_start(xb[:, 4:8, :], xr[:, 4:8, hs])
                        nc.vector.tensor_max(
                            l1[:], xb[:, 0:4, :], xb[:, 4:8, :]
                        )

                    # ACT: w = 1 + 2*tm (fp8 -> fp16), accum: sum w = Fh + 2*tmc
                    w = mtp.tile([P, 7, Fh], _f16, tag="tmf")
                    for ci in range(7):
                        nc.scalar.activation(
                            w[:, ci, :],
                            tm8[:, ci, :],
                            _act.Relu,
                            bias=kbias1[:, 0:1],
                            scale=2.0,
                            accum_out=acc[:, 8 * h + ci : 8 * h + ci + 1],
                        )

                    l2 = mtp.tile([P, 2, Fh], _f16, tag="l2")
                    nc.vector.tensor_max(l2[:], l1[:, 0:2, :], l1[:, 2:4, :])
                    mx = wk.tile([P, Fh], _f16, tag="mx")
                    nc.vector.tensor_max(mx[:], l2[:, 0, :], l2[:, 1, :])

                    pm = mtp.tile([P, 7, Fh], _f16, tag="pm")
                    mxb = mx.rearrange("p (o f) -> p o f", o=1).broadcast_to(
                        (P, 7, Fh)
                    )
                    nc.vector.tensor_tensor(
                        out=pm[:], in0=xb[:, 1:8, :], in1=mxb, op=_alu.is_ge
                    )
                    qm = mtp.tile([P, 7, Fh], _f16, tag="prod")
                    nc.vector.tensor_tensor(
                        out=qm[:], in0=pm[:], in1=w[:], op=_alu.mult
                    )
                    # pmc per class via fast free-dim reduce -> acc cols 24+
                    nc.vector.tensor_reduce(
                        out=acc[:, 24 + 8 * h : 31 + 8 * h],
                        in_=pm[:],
                        axis=mybir.AxisListType.X,
                        op=_alu.add,
                    )

                    nblk = Fh // 512
                    for ci in range(7):
                        for j in range(nblk):
                            js = slice(512 * j, 512 * (j + 1))
                            first = nmm == 0
                            last = nmm == 7 * NBLK - 1
                            nc.tensor.matmul(
                                ppQ[:],
                                ecs[:, ci, :],
                                qm[:, ci, js],
                                start=first,
                                stop=last,
                            )
                            nmm += 1

                # ---- epilogue for sample b ----
                # acc cols 8h+c: sum w = Fh + 2*tmc_chunk; cols 24+8h+c: pmc_chunk
                prQ = acp.tile([8, 1], _f32, tag="prQ")
                ajQ = wk.tile([8, 512], _f32, tag="ajQ")
                nc.scalar.activation(ajQ[:], ppQ[:], _act.Copy, accum_out=prQ[:])

                ptm = ps.tile([1, 48], _f32, tag="ptm", bufs=1)
                nc.tensor.matmul(
                    ptm[:], ones32[:], acc[:, 0:48], start=True, stop=True
                )
                pt3 = ps.tile([1, 8], _f32, tag="pt3", bufs=1)
                nc.tensor.matmul(
                    pt3[0:1, 0:7], prQ[0:7, :], i7[0:7, 0:7], start=True, stop=True
                )

                cntm = wk.tile([1, 48], _f32, tag="cntm")
                nc.scalar.copy(cntm[:], ptm[:])
                wsum = wk.tile([1, 8], _f32, tag="wsum")
                pmc = wk.tile([1, 8], _f32, tag="pmc")
                if NH == 3:
                    tmA = wk.tile([1, 8], _f32, tag="tmA")
                    nc.vector.tensor_add(
                        tmA[0:1, 0:7], cntm[0:1, 0:7], cntm[0:1, 8:15]
                    )
                    nc.vector.tensor_add(
                        wsum[0:1, 0:7], tmA[0:1, 0:7], cntm[0:1, 16:23]
                    )
                    tmB = wk.tile([1, 8], _f32, tag="tmB")
                    nc.vector.tensor_add(
                        tmB[0:1, 0:7], cntm[0:1, 24:31], cntm[0:1, 32:39]
                    )
                    nc.vector.tensor_add(
                        pmc[0:1, 0:7], tmB[0:1, 0:7], cntm[0:1, 40:47]
                    )
                else:
                    nc.vector.tensor_add(
                        wsum[0:1, 0:7], cntm[0:1, 0:7], cntm[0:1, 8:15]
                    )
                    nc.vector.tensor_add(
                        pmc[0:1, 0:7], cntm[0:1, 24:31], cntm[0:1, 32:39]
                    )

                # den = pmc + tmc + EPS = pmc + wsum/2 - F/2 + EPS
                den0 = wk.tile([1, 8], _f32, tag="den0")
                nc.vector.scalar_tensor_tensor(
                    out=den0[0:1, 0:7],
                    in0=wsum[0:1, 0:7],
                    scalar=0.5,
                    in1=pmc[0:1, 0:7],
                    op0=_alu.mult,
                    op1=_alu.add,
                )
                den = wk.tile([1, 8], _f32, tag="den")
                nc.vector.tensor_scalar_add(
                    den[0:1, 0:7], den0[0:1, 0:7], EPS - (P * F) / 2.0
                )
                cnt3 = wk.tile([1, 8], _f32, tag="cnt3")
                nc.scalar.copy(cnt3[0:1, 0:7], pt3[0:1, 0:7])
                # num = sum(qm) - pmc = 2*tps
                num = wk.tile([1, 8], _f32, tag="num")
                nc.vector.tensor_sub(
                    num[0:1, 0:7], cnt3[0:1, 0:7], pmc[0:1, 0:7]
                )
                rec = wk.tile([1, 8], _f32, tag="rec")
                nc.vector.reciprocal(rec[0:1, 0:7], den[0:1, 0:7])
                dice = wk.tile([1, 8], _f32, tag="dice")
                nc.vector.scalar_tensor_tensor(
                    out=dice[0:1, 0:7],
                    in0=num[0:1, 0:7],
                    scalar=1.0 / 7.0,
                    in1=rec[0:1, 0:7],
                    op0=_alu.mult,
                    op1=_alu.mult,
                    accum_out=osb[0:1, b : b + 1],
                )

            nc.sync.dma_start(o_dram[:], osb[:])

    nc.compile()
    return nc


_NC_CACHE = {}


def _get_nc():
    if "nc" not in _NC_CACHE:
        _NC_CACHE["nc"] = _build_nc()
    return _NC_CACHE["nc"]


def make_in_maps(inputs: np.ndarray, targets: np.ndarray) -> list:
    x = (
        np.ascontiguousarray(inputs, dtype=np.float32)
        .astype(np.float16)
        .reshape(NCORES, BPC, C, P, F)
    )
    t = np.ascontiguousarray(targets).reshape(B, H * W)
    oh = t[:, None, :] == np.arange(1, 8, dtype=t.dtype)[None, :, None]
    m = oh.astype(ml_dtypes.float8_e4m3).reshape(NCORES, BPC, 7, P, F)
    eye = np.eye(8, dtype=np.float32)
    return [{"x": x[i], "m": m[i], "i7": eye} for i in range(NCORES)]


def kernel(inputs: np.ndarray, targets: np.ndarray) -> np.ndarray:
    in_maps = make_in_maps(inputs, targets)
    nc = _get_nc()
    res = run_bass_kernel_spmd(nc, in_maps, list(range(NCORES)))
    outs = [res.results[i]["o"].reshape(BPC) for i in range(NCORES)]
    return np.concatenate(outs).astype(np.float32)


# revision 6
# speedup vs baseline: 1.4699x; 1.4699x over previous
"""Dice metric kernel for Trainium2 (Bass/Tile), 8-core data parallel.

Reference computation (per sample b):
    pred = argmax_c logits[b, :, h, w]   (softmax is monotonic -> argmax)
    For classes c = 1..7:
        tps_c  = #{pred == c and tgt == c}
        pmc_c  = #{pred == c},  tmc_c = #{tgt == c}
        dice_c = 2*tps_c / (pmc_c + tmc_c + 1e-5)
    out[b] = mean_c dice_c

Device design (v2): the host supplies a one-hot fp8 encoding of the target
(classes 1..7), which moves all per-class masking off the half-rate
scalar_tensor_tensor path:
  - DVE: max tree (3 batched tensor_max), pm = is_ge(x[1:8], mx) (1 op),
    prod = pm * tm_f16 (1 op). All full-rate tensor_tensor.
  - ACT: tm fp8 -> fp16 convert via Relu, fused accum gives tmc directly.
  - PE:  pmc and tps as ones-style matmul column sums of pm / prod,
    PSUM-accumulated across 512-col blocks and chunks.

Sharding: batch 16 -> 2 samples per core on 8 cores; host concatenates.
"""

import numpy as np
import ml_dtypes

import concourse.bacc as bacc
import concourse.mybir as mybir
import concourse.tile as tile
from concourse.bass_utils import run_bass_kernel_spmd

B, C, H, W = 16, 8, 512, 512
NCORES = 8
BPC = B // NCORES          # samples per core
P = 128                    # SBUF partitions
F = (H * W) // P           # free dim per plane (2048)
EPS = 1e-5

_f32 = mybir.dt.float32
_f16 = mybir.dt.float16
_f8 = mybir.dt.float8e4
_alu = mybir.AluOpType
_act = mybir.ActivationFunctionType


def _build_nc():
    nc = bacc.Bacc(None, target_bir_lowering=False, debug=False)
    x_dram = nc.dram_tensor("x", [BPC, C, P, F], _f16, kind="ExternalInput")
    m_dram = nc.dram_tensor("m", [BPC, 7, P, F], _f8, kind="ExternalInput")
    o_dram = nc.dram_tensor("o", [1, BPC], _f32, kind="ExternalOutput")
    i7_dram = nc.dram_tensor("i7", [8, 8], _f32, kind="ExternalInput")

    with tile.TileContext(nc) as tc:
        with (
            tc.tile_pool(name="xp", bufs=2) as xp,
            tc.tile_pool(name="mt", bufs=2) as mtp,
            tc.tile_pool(name="wk", bufs=2) as wk,
            tc.tile_pool(name="ac", bufs=2) as acp,
            tc.tile_pool(name="cst", bufs=1) as cst,
            tc.tile_pool(name="ps", bufs=2, space="PSUM") as ps,
        ):
            ones32 = cst.tile([P, 1], _f32)
            nc.gpsimd.memset(ones32[:], 1.0)
            kbias0 = cst.tile([P, 1], _f32)
            nc.gpsimd.memset(kbias0[:], 0.0)
            kbias1 = cst.tile([P, 1], _f32)
            nc.gpsimd.memset(kbias1[:], 1.0)
            ecs = cst.tile([P, 7, 8], _f16)
            nc.gpsimd.memset(ecs[:], 0.0)
            for ci in range(7):
                nc.gpsimd.memset(ecs[:, ci, ci : ci + 1], 1.0)
            osb = cst.tile([1, BPC], _f32)
            i7 = cst.tile([8, 8], _f32)

            for b in range(BPC):
                BOUNDS = [0, 512, 1024, F] if b == 0 else [0, 1024, F]
                NH = len(BOUNDS) - 1
                NBLK = F // 512  # total 512-col blocks per sample

                xr = x_dram[b].rearrange("c p f -> p c f")
                mr = m_dram[b].rearrange("c p f -> p c f")

                acc = acp.tile([P, 24], _f32, tag="acc")
                nc.gpsimd.memset(acc[:], 0.0)
                ppQ = ps.tile([8, 512], _f32, tag="ppQ")
                ppP = ps.tile([8, 512], _f32, tag="ppP")

                nmm = 0  # matmul counter for start/stop flags
                for h in range(NH):
                    hs = slice(BOUNDS[h], BOUNDS[h + 1])
                    Fh = BOUNDS[h + 1] - BOUNDS[h]
                    xb = xp.tile([P, C, Fh], _f16, tag="x")
                    tm8 = xp.tile([P, 7, Fh], _f8, tag="tm8")
                    l1 = mtp.tile([P, 4, Fh], _f16, tag="l1")

                    nc.sync.dma_start(tm8[:], mr[:, 0:7, hs])
                    if b == 0 and h == 0:
                        nc.sync.dma_start(xb[:, 0:2, :], xr[:, 0:2, hs])
                        nc.sync.dma_start(xb[:, 4:6, :], xr[:, 4:6, hs])
                        nc.sync.dma_start(xb[:, 2:4, :], xr[:, 2:4, hs])
                        nc.sync.dma_start(xb[:, 6:8, :], xr[:, 6:8, hs])
                        nc.sync.dma_start(i7[:], i7_dram[:])
                        nc.vector.tensor_max(
                            l1[:, 0:2, :], xb[:, 0:2, :], xb[:, 4:6, :]
                        )
                        nc.vector.tensor_max(
                            l1[:, 2:4, :], xb[:, 2:4, :], xb[:, 6:8, :]
                        )
                    else:
                        nc.sync.dma_start(xb[:, 0:4, :], xr[:, 0:4, hs])
                        nc.sync.dma_start(xb[:, 4:8, :], xr[:, 4:8, hs])
                        nc.vector.tensor_max(
                            l1[:], xb[:, 0:4, :], xb[:, 4:8, :]
                        )

                    # ACT: w = 1 + 2*tm (fp8 -> fp16), accum: sum w = Fh + 2*tmc
                    w = mtp.tile([P, 7, Fh], _f16, tag="tmf")
                    for ci in range(7):
                        nc.scalar.activation(
                            w[:, ci, :],
                            tm8[:, ci, :],
                            _act.Relu,
                            bias=kbias1[:, 0:1],
                            scale=2.0,
                            accum_out=acc[:, 8 * h + ci : 8 * h + ci + 1],
                        )

                    l2 = mtp.tile([P, 2, Fh], _f16, tag="l2")
                    nc.vector.tensor_max(l2[:], l1[:, 0:2, :], l1[:, 2:4, :])
                    mx = wk.tile([P, Fh], _f16, tag="mx")
                    nc.vector.tensor_max(mx[:], l2[:, 0, :], l2[:, 1, :])

                    pm = mtp.tile([P, 7, Fh], _f16, tag="pm")
                    mxb = mx.rearrange("p (o f) -> p o f", o=1).broadcast_to(
                        (P, 7, Fh)
                    )
                    nc.vector.tensor_tensor(
                        out=pm[:], in0=xb[:, 1:8, :], in1=mxb, op=_alu.is_ge
                    )
                    qm = mtp.tile([P, 7, Fh], _f16, tag="prod")
                    nc.vector.tensor_tensor(
                        out=qm[:], in0=pm[:], in1=w[:], op=_alu.mult
                    )
                    nblk = Fh // 512
                    for ci in range(7):
                        for j in range(nblk):
                            js = slice(512 * j, 512 * (j + 1))
                            first = nmm == 0
                            last = nmm == 7 * NBLK - 1
                            nc.tensor.matmul(
                                ppP[:],
                                ecs[:, ci, :],
                                pm[:, ci, js],
                                start=first,
                                stop=last,
                            )
                            nc.tensor.matmul(
                                ppQ[:],
                                ecs[:, ci, :],
                                qm[:, ci, js],
                                start=first,
                                stop=last,
                            )
                            nmm += 1

                # ---- epilogue for sample b ----
                # acc cols 8h+c: sum w = Fh + 2*tmc_chunk; cols 24+8h+c: pmc_chunk
                prQ = acp.tile([8, 1], _f32, tag="prQ")
                ajQ = wk.tile([8, 512], _f32, tag="ajQ")
                nc.scalar.activation(ajQ[:], ppQ[:], _act.Copy, accum_out=prQ[:])
                prP = acp.tile([8, 1], _f32, tag="prP")
                ajP = wk.tile([8, 512], _f32, tag="ajP")
                nc.scalar.activation(ajP[:], ppP[:], _act.Copy, accum_out=prP[:])

                ptm = ps.tile([1, 24], _f32, tag="ptm", bufs=1)
                nc.tensor.matmul(
                    ptm[:], ones32[:], acc[:, 0:24], start=True, stop=True
                )
                pt3 = ps.tile([1, 8], _f32, tag="pt3", bufs=1)
                nc.tensor.matmul(
                    pt3[0:1, 0:7], prQ[0:7, :], i7[0:7, 0:7], start=True, stop=True
                )
                pt2 = ps.tile([1, 8], _f32, tag="pt2", bufs=1)
                nc.tensor.matmul(
                    pt2[0:1, 0:7], prP[0:7, :], i7[0:7, 0:7], start=True, stop=True
                )

                cntm = wk.tile([1, 24], _f32, tag="cntm")
                nc.scalar.copy(cntm[:], ptm[:])
                wsum = wk.tile([1, 8], _f32, tag="wsum")
                pmc = wk.tile([1, 8], _f32, tag="pmc")
                if NH == 3:
                    tmA = wk.tile([1, 8], _f32, tag="tmA")
                    nc.vector.tensor_add(
                        tmA[0:1, 0:7], cntm[0:1, 0:7], cntm[0:1, 8:15]
                    )
                    nc.vector.tensor_add(
                        wsum[0:1, 0:7], tmA[0:1, 0:7], cntm[0:1, 16:23]
                    )
                    nc.scalar.copy(pmc[0:1, 0:7], pt2[0:1, 0:7])
                else:
                    nc.vector.tensor_add(
                        wsum[0:1, 0:7], cntm[0:1, 0:7], cntm[0:1, 8:15]
                    )
                    nc.scalar.copy(pmc[0:1, 0:7], pt2[0:1, 0:7])

                # den = pmc + tmc + EPS = pmc + wsum/2 - F/2 + EPS
                den0 = wk.tile([1, 8], _f32, tag="den0")
                nc.vector.scalar_tensor_tensor(
                    out=den0[0:1, 0:7],
                    in0=wsum[0:1, 0:7],
                    scalar=0.5,
                    in1=pmc[0:1, 0:7],
                    op0=_alu.mult,
                    op1=_alu.add,
                )
                den = wk.tile([1, 8], _f32, tag="den")
                nc.vector.tensor_scalar_add(
                    den[0:1, 0:7], den0[0:1, 0:7], EPS - (P * F) / 2.0
                )
                cnt3 = wk.tile([1, 8], _f32, tag="cnt3")
                nc.scalar.copy(cnt3[0:1, 0:7], pt3[0:1, 0:7])
                # num = sum(qm) - pmc = 2*tps
                num = wk.tile([1, 8], _f32, tag="num")
                nc.vector.tensor_sub(
                    num[0:1, 0:7], cnt3[0:1, 0:7], pmc[0:1, 0:7]
                )
                rec = wk.tile([1, 8], _f32, tag="rec")
                nc.vector.reciprocal(rec[0:1, 0:7], den[0:1, 0:7])
                dice = wk.tile([1, 8], _f32, tag="dice")
                nc.vector.scalar_tensor_tensor(
                    out=dice[0:1, 0:7],
                    in0=num[0:1, 0:7],
                    scalar=1.0 / 7.0,
                    in1=rec[0:1, 0:7],
                    op0=_alu.mult,
                    op1=_alu.mult,
                    accum_out=osb[0:1, b : b + 1],
                )

            nc.sync.dma_start(o_dram[:], osb[:])

    nc.compile()
    return nc


_NC_CACHE = {}


def _get_nc():
    if "nc" not in _NC_CACHE:
        _NC_CACHE["nc"] = _build_nc()
    return _NC_CACHE["nc"]


def make_in_maps(inputs: np.ndarray, targets: np.ndarray) -> list:
    x = (
        np.ascontiguousarray(inputs, dtype=np.float32)
        .astype(np.float16)
        .reshape(NCORES, BPC, C, P, F)
    )
    t = np.ascontiguousarray(targets).reshape(B, H * W)
    oh = t[:, None, :] == np.arange(1, 8, dtype=t.dtype)[None, :, None]
    m = oh.astype(ml_dtypes.float8_e4m3).reshape(NCORES, BPC, 7, P, F)
    eye = np.eye(8, dtype=np.float32)
    return [{"x": x[i], "m": m[i], "i7": eye} for i in range(NCORES)]


def kernel(inputs: np.ndarray, targets: np.ndarray) -> np.ndarray:
    in_maps = make_in_maps(inputs, targets)
    nc = _get_nc()
    res = run_bass_kernel_spmd(nc, in_maps, list(range(NCORES)))
    outs = [res.results[i]["o"].reshape(BPC) for i in range(NCORES)]
    return np.concatenate(outs).astype(np.float32)
